# revision 1
# baseline (speedup 1.0000x reference)
"""Trainium2 Bass kernel for quantized attention (nn_Attention_own_quan).

Full-input contract: kernel(**inputs) takes the unsharded inputs and returns
the full output. Internally shards (batch, head-group) across 8 NeuronCores:
core c handles batch c//2 and heads [4*(c%2), 4*(c%2)+4).

All fake-quantization (clamp(round(x/s))*s) is done on-device with exact
round-half-to-even via the +1.5*2^23 magic-constant trick; quantized integer
values are carried in bf16 (exact for |v|<=256) so the tensor engine can
matmul them; integer dot products accumulate exactly in fp32 PSUM and are
rescaled by combined quantization scales. The softmax probabilities are
transposed for the p@v contraction with the DMA xbar transpose.
"""

import sys

sys.path.insert(0, "/opt/trn_rl_repo")

import numpy as np

import concourse.bacc as bacc
import concourse.mybir as mybir
import concourse.tile as tile
from concourse.bass_utils import run_bass_kernel_spmd

F32 = mybir.dt.float32
BF16 = mybir.dt.bfloat16
AF = mybir.ActivationFunctionType
OP = mybir.AluOpType

B, S, D = 4, 2048, 512
H, DH = 8, 64
N_CORES = 8
HPC = H // 2          # heads per core = 4
EPC = HPC * DH        # head-dim columns per core = 256
MAGIC = float(np.float32(12582912.0))  # 1.5 * 2**23: round-to-nearest-even trick

_prog_cache = {}


def _build(consts):
    """Build the single-core Bass/Tile program (SPMD across 8 cores)."""
    (rs0, rswq, rswk, rswv, rswo, cq, ck, cv, ce, wclamp, cx, cout) = consts

    nc = bacc.Bacc("TRN2", target_bir_lowering=False, debug=False)

    hsT = nc.declare_dram_parameter("hsT", [D, S], F32, isOutput=False)
    wqT = nc.declare_dram_parameter("wqT", [D, EPC], F32, isOutput=False)
    wkT = nc.declare_dram_parameter("wkT", [D, EPC], F32, isOutput=False)
    wvT = nc.declare_dram_parameter("wvT", [D, EPC], F32, isOutput=False)
    woT = nc.declare_dram_parameter("woT", [EPC, D], F32, isOutput=False)
    outT = nc.declare_dram_parameter("outT", [D, S], F32, isOutput=True)

    DT = D // 128      # 4 d-tiles
    ET = EPC // 128    # 2 e-tiles
    ST = S // 128      # 16 s-tiles
    SC = S // 512      # 4 512-chunks
    NG = ST // 2       # 8 pair-groups of s_q blocks

    with tile.TileContext(nc) as tc:
        with (
            tc.tile_pool(name="persist", bufs=1) as persist,
            tc.tile_pool(name="wstage", bufs=1) as wstage,
            tc.tile_pool(name="hstage", bufs=1) as hstage,
            tc.tile_pool(name="work", bufs=4) as work,
            tc.tile_pool(name="pwork", bufs=3) as pwork,
            tc.tile_pool(name="ptwork", bufs=7) as ptwork,
            tc.tile_pool(name="ptmp", bufs=2) as ptmp,
            tc.tile_pool(name="outst", bufs=1) as outst,
            tc.tile_pool(name="small", bufs=8) as small,
            tc.tile_pool(name="ps_mm", bufs=2, space="PSUM") as ps_mm,
            tc.tile_pool(name="ps_s", bufs=3, space="PSUM") as ps_s,
        ):
            # ---- weight quantization (q/k now; v/o deferred) ----
            def quant_weight(dram, rs, shape, tag):
                kt = shape[0] // 128
                st_ = wstage.tile([128, kt, shape[1]], F32, tag="wst")
                nc.gpsimd.dma_start(
                    out=st_[:], in_=dram.rearrange("(t p) e -> p t e", p=128)
                )
                tmp = wstage.tile([128, kt, shape[1]], F32, tag="wtmp")
                if wclamp:
                    nc.vector.tensor_scalar(
                        out=tmp[:], in0=st_[:], scalar1=rs, scalar2=127.0,
                        op0=OP.mult, op1=OP.min,
                    )
                    nc.vector.tensor_scalar(
                        out=tmp[:], in0=tmp[:], scalar1=-128.0, scalar2=MAGIC,
                        op0=OP.max, op1=OP.add,
                    )
                else:
                    nc.vector.tensor_scalar(
                        out=tmp[:], in0=st_[:], scalar1=rs, scalar2=MAGIC,
                        op0=OP.mult, op1=OP.add,
                    )
                wi = persist.tile([128, kt, shape[1]], BF16, tag=tag)
                nc.vector.tensor_scalar(
                    out=wi[:], in0=tmp[:], scalar1=MAGIC, scalar2=None,
                    op0=OP.subtract,
                )
                return wi

            wq_i = quant_weight(wqT, rswq, (D, EPC), "wq_i")
            wk_i = quant_weight(wkT, rswk, (D, EPC), "wk_i")

            # ---- hs quantization: alternate tiles between DVE and GPSIMD ----
            hsq = persist.tile([128, DT, S], BF16, tag="hsq")
            for t in range(DT):
                hst = hstage.tile([128, S], F32, tag="hst")
                nc.gpsimd.dma_start(
                    out=hst[:],
                    in_=hsT.rearrange("(t p) s -> p t s", p=128)[:, t, :],
                )
                eng = nc.vector
                t1 = work.tile([128, S], F32, tag="e")
                eng.tensor_scalar(
                    out=t1[:], in0=hst[:], scalar1=rs0, scalar2=127.0,
                    op0=OP.mult, op1=OP.min,
                )
                eng.tensor_scalar(
                    out=t1[:], in0=t1[:], scalar1=-128.0, scalar2=MAGIC,
                    op0=OP.max, op1=OP.add,
                )
                eng.tensor_scalar(
                    out=hsq[:, t, :], in0=t1[:], scalar1=MAGIC, scalar2=None,
                    op0=OP.subtract,
                )

            # ---- q/k projections ----
            qT_b = persist.tile([128, ET, S], BF16, tag="qT_b")
            kT_b = persist.tile([128, ET, S], BF16, tag="kT_b")
            v_b = persist.tile([128, ST, EPC], BF16, tag="v_b")

            def qkv_chunk(wi, cs, dst_slice, psum_w, lhsT_list, rhs_list):
                pq = ps_mm.tile([128, 512], F32, tag="mm")
                nk = len(lhsT_list)
                for kt in range(nk):
                    nc.tensor.matmul(
                        pq[:, :psum_w], lhsT_list[kt], rhs_list[kt],
                        start=(kt == 0), stop=(kt == nk - 1),
                    )
                tq = ptmp.tile([128, 512], F32, tag="ptmp")
                nc.vector.tensor_scalar(
                    out=tq[:, :psum_w], in0=pq[:, :psum_w], scalar1=cs,
                    scalar2=127.0, op0=OP.mult, op1=OP.min,
                )
                nc.vector.tensor_scalar(
                    out=tq[:, :psum_w], in0=tq[:, :psum_w], scalar1=-128.0,
                    scalar2=MAGIC, op0=OP.max, op1=OP.add,
                )
                nc.vector.tensor_scalar(
                    out=dst_slice, in0=tq[:, :psum_w],
                    scalar1=MAGIC, scalar2=None, op0=OP.subtract,
                )

            for mt in range(ET):
                for wi, cs, dst in ((wq_i, cq, qT_b), (wk_i, ck, kT_b)):
                    for nn in range(SC):
                        qkv_chunk(
                            wi, cs, dst[:, mt, nn * 512:(nn + 1) * 512], 512,
                            [wi[:, kt, mt * 128:(mt + 1) * 128] for kt in range(DT)],
                            [hsq[:, kt, nn * 512:(nn + 1) * 512] for kt in range(DT)],
                        )

            # ---- v/o weights + v projection (needed a bit later) ----
            wv_i = quant_weight(wvT, rswv, (D, EPC), "wv_i")
            wo_i = quant_weight(woT, rswo, (EPC, D), "wo_i")
            for st_i in range(ST):
                qkv_chunk(
                    wv_i, cv, v_b[:, st_i, :], EPC,
                    [hsq[:, kt, st_i * 128:(st_i + 1) * 128] for kt in range(DT)],
                    [wv_i[:, kt, :] for kt in range(DT)],
                )

            # ---- attention ----
            xTf = persist.tile([128, ET, S], F32, tag="xTf")


            def emit_pv_pair(hA, g, ptA, ptB):
                # heads hA (partitions 0:64) and hA+1 (64:128) concurrently
                # via tensor-engine column groups; shared PSUM accumulator.
                mt = hA // 2
                po = ps_mm.tile([128, 256], F32, tag="mm")
                for tt in range(ST):
                    nc.tensor.matmul(
                        po[0:64, :],
                        v_b[:, tt, 64 * hA:64 * hA + 64],
                        ptA[:, tt, :],
                        start=(tt == 0), stop=(tt == ST - 1),
                        tile_position=(0, 0), skip_group_check=True,
                    )
                    nc.tensor.matmul(
                        po[64:128, :],
                        v_b[:, tt, 64 * hA + 64:64 * hA + 128],
                        ptB[:, tt, :],
                        start=(tt == 0), stop=(tt == ST - 1),
                        tile_position=(0, 64), skip_group_check=True,
                    )
                nc.scalar.activation(
                    out=xTf[:, mt, g * 256:(g + 1) * 256],
                    in_=po[:], func=AF.Identity, bias=0.0, scale=cx,
                )

            def emit_tail(ee, r255, pt2, sqi):
                nc.gpsimd.tensor_scalar(
                    out=ee[:], in0=ee[:], scalar1=r255[:], scalar2=MAGIC,
                    op0=OP.mult, op1=OP.add,
                )
                pp = pwork.tile([128, S], BF16, tag="pp")
                nc.vector.tensor_scalar(
                    out=pp[:], in0=ee[:], scalar1=MAGIC, scalar2=127.0,
                    op0=OP.subtract, op1=OP.min,
                )
                nc.sync.dma_start(
                    out=pt2[:, :, sqi * 128:(sqi + 1) * 128],
                    in_=pp[:], transpose=True,
                )

            chainq = []            # one-block deferred normalize/transpose
            ptmap = {}             # (h, g) -> pt2 tile
            for g in range(NG):
                for hA in (0, 2):  # head pairs (0,1) and (2,3)
                    mt = hA // 2
                    ptA = ptwork.tile([128, ST, 256], BF16, tag="pt")
                    ptB = ptwork.tile([128, ST, 256], BF16, tag="pt")
                    ptmap[(hA, g)] = ptA
                    ptmap[(hA + 1, g)] = ptB
                    for sqi in range(2):
                        sq = g * 2 + sqi
                        eeA = work.tile([128, S], F32, tag="e")
                        eeB = work.tile([128, S], F32, tag="e")
                        sums = {0: [], 1: []}
                        for half in range(2):
                            # row-group packed: head hA on array rows 0-63,
                            # head hA+1 on rows 64-127, running concurrently
                            pssA = ps_s.tile([128, 1024], F32, tag="sc")
                            pssB = ps_s.tile([128, 1024], F32, tag="sc")
                            for ckk in range(2):
                                nn = half * 2 + ckk
                                nc.tensor.matmul(
                                    pssA[:, ckk * 512:(ckk + 1) * 512],
                                    qT_b[0:64, mt, sq * 128:(sq + 1) * 128],
                                    kT_b[0:64, mt, nn * 512:(nn + 1) * 512],
                                    start=True, stop=True,
                                    tile_position=(0, 0),
                                )
                                nc.tensor.matmul(
                                    pssB[:, ckk * 512:(ckk + 1) * 512],
                                    qT_b[64:128, mt, sq * 128:(sq + 1) * 128],
                                    kT_b[64:128, mt, nn * 512:(nn + 1) * 512],
                                    start=True, stop=True,
                                    tile_position=(64, 0),
                                )
                            for i, (pss, ee) in enumerate(((pssA, eeA), (pssB, eeB))):
                                sh = small.tile([128, 1], F32, tag="sh")
                                nc.scalar.activation(
                                    out=ee[:, half * 1024:(half + 1) * 1024],
                                    in_=pss[:], func=AF.Exp,
                                    bias=0.0, scale=ce, accum_out=sh[:],
                                )
                                sums[i].append(sh)
                        for i, (ee, pt2) in enumerate(((eeA, ptA), (eeB, ptB))):
                            ss = small.tile([128, 1], F32, tag="ss")
                            nc.vector.tensor_add(ss[:], sums[i][0][:], sums[i][1][:])
                            rr = small.tile([128, 1], F32, tag="rr")
                            nc.vector.reciprocal(rr[:], ss[:])
                            r255 = small.tile([128, 1], F32, tag="r255")
                            nc.vector.tensor_scalar(
                                out=r255[:], in0=rr[:], scalar1=255.0,
                                scalar2=None, op0=OP.mult,
                            )
                            chainq.append((ee, r255, pt2, sqi))
                            if len(chainq) >= 2:
                                emit_tail(*chainq.pop(0))
                    # deferred p@v pair of the previous group
                    if g >= 1:
                        emit_pv_pair(hA, g - 1, ptmap.pop((hA, g - 1)),
                                     ptmap.pop((hA + 1, g - 1)))
            while chainq:
                emit_tail(*chainq.pop(0))
            emit_pv_pair(0, NG - 1, ptmap.pop((0, NG - 1)), ptmap.pop((1, NG - 1)))
            emit_pv_pair(2, NG - 1, ptmap.pop((2, NG - 1)), ptmap.pop((3, NG - 1)))

            # ---- quantize x^T ----
            xTb = persist.tile([128, ET, S], BF16, tag="xTb")
            for mt in range(ET):
                t3 = work.tile([128, S], F32, tag="e")
                nc.vector.tensor_scalar(
                    out=t3[:], in0=xTf[:, mt, :], scalar1=127.0, scalar2=-128.0,
                    op0=OP.min, op1=OP.max,
                )
                nc.vector.tensor_scalar(
                    out=xTb[:, mt, :], in0=t3[:], scalar1=MAGIC, scalar2=MAGIC,
                    op0=OP.add, op1=OP.subtract,
                )

            # ---- output projection ----
            for mt in range(DT):
                ot = outst.tile([128, S], F32, tag="ot")
                for nn in range(SC):
                    pf = ps_mm.tile([128, 512], F32, tag="mm")
                    for kt in range(ET):
                        nc.tensor.matmul(
                            pf[:],
                            wo_i[:, kt, mt * 128:(mt + 1) * 128],
                            xTb[:, kt, nn * 512:(nn + 1) * 512],
                            start=(kt == 0), stop=(kt == ET - 1),
                        )
                    if nn % 2 == 0:
                        nc.scalar.activation(
                            out=ot[:, nn * 512:(nn + 1) * 512], in_=pf[:],
                            func=AF.Identity, bias=0.0, scale=cout,
                        )
                    else:
                        nc.vector.tensor_scalar(
                            out=ot[:, nn * 512:(nn + 1) * 512], in0=pf[:],
                            scalar1=cout, scalar2=None, op0=OP.mult,
                        )
                nc.gpsimd.dma_start(
                    out=outT[mt * 128:(mt + 1) * 128, :], in_=ot[:],
                )

    nc.finalize()
    return nc


def kernel(hs, Wq, Wk, Wv, Wo, bo, scales, **_ignored):
    hs = np.asarray(hs, dtype=np.float32)
    Wq = np.asarray(Wq, dtype=np.float32)
    Wk = np.asarray(Wk, dtype=np.float32)
    Wv = np.asarray(Wv, dtype=np.float32)
    Wo = np.asarray(Wo, dtype=np.float32)
    bo = np.asarray(bo, dtype=np.float32)
    sc = np.asarray(scales, dtype=np.float32)

    one = np.float32(1.0)
    # The reference requantizes hs by s1/s3/s5 after quantizing by s0; with
    # s1 == s3 == s5 == s0 (as set up) that is an exact no-op on the integers.
    assert np.allclose(sc[1], sc[0]) and np.allclose(sc[3], sc[0]) and np.allclose(sc[5], sc[0])
    assert np.allclose(sc[9], one / np.float32(255.0)) and np.allclose(sc[10], sc[9])

    consts = (
        float(one / sc[0]),                       # rs0
        float(one / sc[2]),                       # rswq
        float(one / sc[4]),                       # rswk
        float(one / sc[6]),                       # rswv
        float(one / sc[13]),                      # rswo
        float(sc[1] * sc[2] / sc[7]),             # cq
        float(sc[3] * sc[4] / sc[8]),             # ck
        float(sc[5] * sc[6] / sc[11]),            # cv
        float(sc[7] * sc[8] * np.float32(DH ** -0.5)),  # ce
        bool(max(
            float(np.abs(Wq).max() / sc[2]), float(np.abs(Wk).max() / sc[4]),
            float(np.abs(Wv).max() / sc[6]), float(np.abs(Wo).max() / sc[13]),
        ) > 126.49),                              # wclamp needed?
        float(sc[11] / np.float32(255.0) / sc[12]),     # cx
        float(sc[12] * sc[13]),                   # cout
    )

    if consts not in _prog_cache:
        _prog_cache[consts] = _build(consts)
    nc = _prog_cache[consts]

    in_maps = []
    for c in range(N_CORES):
        b = c // 2
        g = c % 2
        es = slice(g * EPC, (g + 1) * EPC)
        in_maps.append({
            "hsT": np.ascontiguousarray(hs[b].T),
            "wqT": np.ascontiguousarray(Wq.T[:, es]),
            "wkT": np.ascontiguousarray(Wk.T[:, es]),
            "wvT": np.ascontiguousarray(Wv.T[:, es]),
            "woT": np.ascontiguousarray(Wo.T[es, :]),
        })

    res = run_bass_kernel_spmd(nc, in_maps, list(range(N_CORES)))
    outs = res.results

    out = np.empty((B, S, D), dtype=np.float32)
    for b in range(B):
        acc = outs[2 * b]["outT"] + outs[2 * b + 1]["outT"]
        out[b] = acc.T + bo[None, :]
    return out

